# revision 1
# baseline (speedup 1.0000x reference)
"""CantorAttention Trainium2 kernel.

Problem (hardcoded): B=2, S=2048, DIM=512, H=8 heads, D=64, K=64 routes.
  qkv = x @ w_qkv + b_qkv ; per-head sparse attention over routes[q, :] ;
  out = attn_out @ w_out + b_out.

Strategy (8 cores): shard batch x head-pairs. Core i handles batch i//4 and
heads (2*(i%4), 2*(i%4)+1). Routes are shared across batch/heads, so the
sparse attention is run DENSE on the TensorEngine with a host-precomputed
multiplicative count-mask C^T[k, q] = #{j : routes[q, j] == k}:

  P[k, q]  = C^T[k, q] * exp(scale * (K q_vec . k_vec))       (0 off-route)
  out_h    = (V^T_aug @ P) / denom,  denom from an appended ones-column
  partial  = concat_h(out_h) @ w_out[head rows]               (per core)

Host gathers: final[b] = sum of the 4 partials of batch b + b_out.
Exactly reproduces softmax over the 64 routed scores (duplicates included
via the count mask).

Everything on PE is bf16 with fp32 PSUM accumulation; exp on ScalarE;
mask-multiply on VectorE (bf16 2x mode); transposed layouts throughout so
no on-chip transposes are needed except V (one PE transpose per key tile).
"""

import numpy as np
import ml_dtypes

import concourse.bass as bass
import concourse.bacc as bacc
import concourse.mybir as mybir
import concourse.tile as tile
from concourse.bass_utils import run_bass_kernel_spmd
from concourse.masks import make_identity

BF16 = mybir.dt.bfloat16
F32 = mybir.dt.float32
NPBF16 = ml_dtypes.bfloat16

B = 2
S = 2048
DIM = 512
H = 8
D = 64
KR = 64
SCALE = 0.125

P = 128
NKT = S // P      # 16 key tiles
QC = 512          # query chunk (psum bank width)
NQC = S // QC     # 4 query chunks
NC4 = DIM // P    # 4 contraction chunks

_CACHE = {}


def build_nc():
    if "nc" in _CACHE:
        return _CACHE["nc"]
    nc = bacc.Bacc(
        "TRN2",
        target_bir_lowering=False,
        debug=False,
        num_devices=8,
    )

    xt_d = nc.dram_tensor("xt", [P, NC4 * S], BF16, kind="ExternalInput").ap()
    wq_d = nc.dram_tensor("wq", [P, NC4 * P], BF16, kind="ExternalInput").ap()
    wk_d = nc.dram_tensor("wk", [P, NC4 * P], BF16, kind="ExternalInput").ap()
    wv_d = nc.dram_tensor("wv", [P, NC4 * P], BF16, kind="ExternalInput").ap()
    bq_d = nc.dram_tensor("bq", [P, 1], F32, kind="ExternalInput").ap()
    bk_d = nc.dram_tensor("bk", [P, 1], F32, kind="ExternalInput").ap()
    bv_d = nc.dram_tensor("bv", [P, 1], F32, kind="ExternalInput").ap()
    ct_d = nc.dram_tensor("ct", [P, NKT * S], BF16, kind="ExternalInput").ap()
    wo_d = nc.dram_tensor("wo", [P, DIM], BF16, kind="ExternalInput").ap()
    out_d = nc.dram_tensor("out", [S, DIM], F32, kind="ExternalOutput").ap()

    with tile.TileContext(nc) as tc:
        with tc.tile_pool(name="persist", bufs=1) as pp:
            ident = pp.tile([P, P], BF16, tag="ident")
            make_identity(nc, ident[:])

            xt_big = pp.tile([P, NC4 * S], BF16, tag="xtb", name="xt_big")
            nc.sync.dma_start(out=xt_big[:], in_=xt_d[:, :])
            xt_sb = [xt_big[:, c * S:(c + 1) * S] for c in range(NC4)]

            w_sb = {}
            for name, wd in (("q", wq_d), ("k", wk_d), ("v", wv_d)):
                wt = pp.tile([P, NC4 * P], BF16, tag=f"w{name}b", name=f"w{name}_big")
                nc.sync.dma_start(out=wt[:], in_=wd[:, :])
                for c in range(NC4):
                    w_sb[(name, c)] = wt[:, c * P:(c + 1) * P]
            b_sb = {}
            for name, bd in (("q", bq_d), ("k", bk_d), ("v", bv_d)):
                t = pp.tile([P, 1], F32, tag=f"b{name}", name=f"b{name}_sb")
                nc.sync.dma_start(out=t[:], in_=bd[:, :])
                b_sb[name] = t

            wo_sb = pp.tile([P, DIM], BF16, tag="wo")
            nc.sync.dma_start(out=wo_sb[:], in_=wo_d[:, :])
            sel_sb = {}
            for h in range(2):
                t = pp.tile([P, P], F32, tag=f"sel{h}", name=f"sel{h}")
                nc.vector.memset(t[:], 0.0)
                nc.vector.memset(t[0:1, h * D:(h + 1) * D], 1.0)
                sel_sb[h] = t

            ct_big = pp.tile([P, NKT * S], BF16, tag="ctb", name="ct_big")
            nc.sync.dma_start(out=ct_big[:], in_=ct_d[:, :])
            ct_sb = [ct_big[:, kt * S:(kt + 1) * S] for kt in range(NKT)]

            # v^T stacked (2 heads); q^T/k^T per-head, rows 64-127 zero-padded
            # so every main-loop matmul is a full [128,128] stationary operand.
            qkvt = {}
            qkvt["v"] = pp.tile([P, S], BF16, tag="vt", name="vt")
            for name in ("q", "k"):
                for h in range(2):
                    t = pp.tile([P, S], BF16, tag=f"{name}t{h}", name=f"{name}t{h}")
                    nc.vector.memset(t[D:P, :], 0.0)
                    qkvt[(name, h)] = t

            # Phase 1: QKV^T = W^T @ X^T (+bias), bf16.
            with tc.tile_pool(name="psum_pre", bufs=4, space="PSUM") as pre:
                for name in ("k", "q", "v"):
                    for qc in range(NQC):
                        ps = pre.tile([P, QC], F32, tag="qkvps", name="qkvps")
                        for c in range(NC4):
                            nc.tensor.matmul(
                                ps[:],
                                lhsT=w_sb[(name, c)],
                                rhs=xt_sb[c][:, qc * QC:(qc + 1) * QC],
                                start=(c == 0),
                                stop=(c == NC4 - 1),
                            )
                        if name == "v":
                            nc.vector.tensor_tensor(
                                out=qkvt["v"][:, qc * QC:(qc + 1) * QC],
                                in0=ps[:],
                                in1=b_sb["v"][:].to_broadcast([P, QC]),
                                op=mybir.AluOpType.add,
                            )
                        else:
                            for h in range(2):
                                hd = h * D
                                nc.vector.tensor_tensor(
                                    out=qkvt[(name, h)][0:D, qc * QC:(qc + 1) * QC],
                                    in0=ps[hd:hd + D, :],
                                    in1=b_sb[name][hd:hd + D, :].to_broadcast([D, QC]),
                                    op=mybir.AluOpType.add,
                                )

                # Phase 1b: V tiles in [key, d] layout with ones column.
                v_sb = {}
                for h in range(2):
                    for kt in range(NKT):
                        v_sb[(h, kt)] = pp.tile([P, P], BF16, tag=f"v{h}_{kt}", name=f"v{h}_{kt}")
                for kt in range(NKT):
                    tp = pre.tile([P, P], BF16, tag="vtps", name="vtps")
                    nc.tensor.transpose(
                        out=tp[:], in_=qkvt["v"][:, kt * P:(kt + 1) * P],
                        identity=ident[:],
                    )
                    for h in range(2):
                        nc.scalar.copy(
                            out=v_sb[(h, kt)][:, 0:D], in_=tp[:, h * D:(h + 1) * D]
                        )
                        nc.vector.memset(v_sb[(h, kt)][:, D:D + 1], 1.0)
                        nc.vector.memset(v_sb[(h, kt)][:, D + 1:P], 0.0)

            ot_sb = pp.tile([P, S], F32, tag="ot")
            den_sb = {}
            for h in range(2):
                den_sb[h] = pp.tile([P, S], F32, tag=f"den{h}", name=f"den{h}")
                nc.vector.memset(den_sb[h][D:P, :], 0.0)
                nc.vector.memset(den_sb[h][0:D, :], 0.0)
            r2r_sb = pp.tile([P, S], F32, tag="r2r")
            on_sb = pp.tile([P, S], BF16, tag="on")

            # Phase 2: dense masked attention, one head at a time.
            QH = 1024
            for h in range(2):
                hd = h * D
                with tc.tile_pool(name=f"psum_s{h}", bufs=2, space="PSUM") as sp, \
                     tc.tile_pool(name=f"psum_ot{h}", bufs=1, space="PSUM") as op, \
                     tc.tile_pool(name=f"pwork{h}", bufs=6) as pw:
                    ot_ps = op.tile([P, S], F32, tag="otps", name="otps")
                    for kt in range(NKT):
                        for q2 in range(S // QH):
                            s_ps = sp.tile([P, QH], F32, tag="s", name="s_ps")
                            for half in range(QH // QC):
                                off = q2 * QH + half * QC
                                nc.tensor.matmul(
                                    s_ps[:, half * QC:(half + 1) * QC],
                                    lhsT=qkvt[("k", h)][:, kt * P:(kt + 1) * P],
                                    rhs=qkvt[("q", h)][:, off:off + QC],
                                    start=True,
                                    stop=True,
                                )
                            p_sb = pw.tile([P, QH], BF16, tag="p", name="p_sb")
                            nc.scalar.activation(
                                p_sb[:], s_ps[:], mybir.ActivationFunctionType.Exp
                            )
                            pm_sb = pw.tile([P, QH], BF16, tag="pm", name="pm_sb")
                            nc.vector.tensor_tensor(
                                out=pm_sb[:],
                                in0=p_sb[:],
                                in1=ct_sb[kt][:, q2 * QH:(q2 + 1) * QH],
                                op=mybir.AluOpType.mult,
                            )
                            for half in range(QH // QC):
                                off = q2 * QH + half * QC
                                nc.tensor.matmul(
                                    ot_ps[:, off:off + QC],
                                    lhsT=v_sb[(h, kt)][:],
                                    rhs=pm_sb[:, half * QC:(half + 1) * QC],
                                    start=(kt == 0),
                                    stop=(kt == NKT - 1),
                                )
                    nc.scalar.copy(out=ot_sb[hd:hd + D, :], in_=ot_ps[0:D, :])
                    nc.vector.tensor_copy(out=den_sb[h][0:1, :], in_=ot_ps[D:D + 1, :])

            # Phase 3: normalize, project, store (pipelined per 512-chunk).
            with tc.tile_pool(name="psum_r2", bufs=2, space="PSUM") as rp, \
                 tc.tile_pool(name="psum_fin", bufs=3, space="PSUM") as fp, \
                 tc.tile_pool(name="fin_sb", bufs=4) as fsb:
                r2_list = []
                for qc in range(NQC):
                    qs = slice(qc * QC, (qc + 1) * QC)
                    r2_ps = rp.tile([P, QC], F32, tag="r2", name="r2_ps", bufs=4)
                    for h in range(2):
                        nc.tensor.matmul(
                            r2_ps[:],
                            lhsT=sel_sb[h][:],
                            rhs=den_sb[h][:, qs],
                            start=(h == 0),
                            stop=(h == 1),
                        )
                    r2_list.append(r2_ps)
                for qc in range(NQC):
                    qs = slice(qc * QC, (qc + 1) * QC)
                    nc.vector.reciprocal_approx_fast(out=r2r_sb[:, qs], in_=r2_list[qc][:])
                    nc.vector.tensor_tensor(
                        out=on_sb[:, qs], in0=ot_sb[:, qs], in1=r2r_sb[:, qs],
                        op=mybir.AluOpType.mult,
                    )
                    for qt in range(qc * NC4, (qc + 1) * NC4):
                        pr = fp.tile([P, DIM], F32, tag="pr", name="pr_ps")
                        nc.tensor.matmul(
                            pr[:],
                            lhsT=on_sb[:, qt * P:(qt + 1) * P],
                            rhs=wo_sb[:],
                            start=True,
                            stop=True,
                        )
                        o_sb = fsb.tile([P, DIM], F32, tag="osb", name="o_sb")
                        nc.scalar.copy(out=o_sb[:], in_=pr[:])
                        nc.sync.dma_start(
                            out=out_d[qt * P:(qt + 1) * P, :], in_=o_sb[:]
                        )

    nc.compile()
    _CACHE["nc"] = nc
    return nc


def make_in_maps(x, routes, w_qkv, b_qkv, w_out):
    x = np.asarray(x, np.float32)
    routes = np.asarray(routes)
    w_qkv = np.asarray(w_qkv, np.float32)
    b_qkv = np.asarray(b_qkv, np.float32)
    w_out = np.asarray(w_out, np.float32)

    C = np.zeros((S, S), np.float32)
    np.add.at(C, (np.arange(S)[:, None], routes), 1.0)

    def pack(a):
        # [n*128, X] -> [128, n*X]
        n = a.shape[0] // P
        return np.ascontiguousarray(
            a.reshape(n, P, a.shape[1]).transpose(1, 0, 2).reshape(P, -1))

    xt = [pack(np.ascontiguousarray(x[b].T)).astype(NPBF16) for b in range(B)]
    ctp = pack(np.ascontiguousarray(C.T)).astype(NPBF16)

    in_maps = []
    for core in range(8):
        b = core // 4
        hp = core % 4
        col = hp * P
        wq = pack(w_qkv[:, col:col + P] * SCALE).astype(NPBF16)
        wk = pack(w_qkv[:, DIM + col:DIM + col + P]).astype(NPBF16)
        wv = pack(w_qkv[:, 2 * DIM + col:2 * DIM + col + P]).astype(NPBF16)
        bq = (b_qkv[col:col + P] * SCALE).astype(np.float32).reshape(P, 1)
        bk = b_qkv[DIM + col:DIM + col + P].astype(np.float32).reshape(P, 1)
        bv = b_qkv[2 * DIM + col:2 * DIM + col + P].astype(np.float32).reshape(P, 1)
        wo = np.ascontiguousarray(w_out[col:col + P, :]).astype(NPBF16)
        in_maps.append(dict(
            xt=xt[b], wq=wq, wk=wk, wv=wv, bq=bq, bk=bk, bv=bv,
            ct=ctp, wo=wo,
        ))
    return in_maps


def run(inputs, trace=False, trace_cores=None):
    nc = build_nc()
    in_maps = make_in_maps(
        inputs["x"], inputs["routes"], inputs["w_qkv"], inputs["b_qkv"],
        inputs["w_out"],
    )
    res = run_bass_kernel_spmd(
        nc, in_maps, list(range(8)), trace=trace, trace_cores=trace_cores,
    )
    b_out = np.asarray(inputs["b_out"], np.float32)
    final = np.zeros((B, S, DIM), np.float32)
    for core in range(8):
        final[core // 4] += res.results[core]["out"]
    final += b_out[None, None, :]
    return final, res


def kernel(**inputs):
    final, _ = run(inputs, trace=False)
    return final



# revision 3
# speedup vs baseline: 1.6043x; 1.6043x over previous
"""CantorAttention Trainium2 kernel — banded block-sparse edition.

Problem (hardcoded): B=2, S=2048, DIM=512, H=8 heads, D=64, K=64 routes.
  qkv = x @ w_qkv + b_qkv ; per-head sparse attention over routes[q, :] ;
  out = attn_out @ w_out + b_out.

Strategy (8 cores): shard batch x head-pairs. Core i handles batch i//4 and
heads (2*(i%4), 2*(i%4)+1). Routes are shared across batch/heads.

The sparse attention is made BANDED by a host-computed permutation rho of
the sequence (barycenter seriation of the bipartite route graph — for the
Cantor routes this recovers coordinate order and every query's 64 routes
land in a <=128-wide window of permuted key space). Queries and keys are
both processed in rho order; only the nonzero 128x128 (key-tile x
query-tile) blocks of the routed score matrix are computed:

  P[k, q]  = C~[k, q] * exp(scale * (k_vec . q_vec))   (count mask, 0 off-route)
  AV_h     = [V_h | 1]^T @ P      -> rows 0..63 out, row 64 = denominator
  out_h    = AV_h / denom + bv_h  ;  partial = concat_h(out_h)^T @ w_out
Host gathers: final[b][perm] += partial_core ; final += b_out.

Exact softmax over the 64 routed scores for ANY routes input (block list
is derived from the actual routes; a bad permutation only costs speed).
All matmuls bf16 with fp32 PSUM accumulation; exp on ScalarE; mask-multiply
on VectorE; no on-chip transposes anywhere (V is produced directly in
[key, d] layout by using x^T blocks as the stationary operand).
"""

import numpy as np
import ml_dtypes

import concourse.bass as bass
import concourse.bacc as bacc
import concourse.mybir as mybir
import concourse.tile as tile
from concourse.bass_utils import run_bass_kernel_spmd

BF16 = mybir.dt.bfloat16
F32 = mybir.dt.float32
FP16 = mybir.dt.float16
NPBF16 = ml_dtypes.bfloat16

B = 2
S = 2048
DIM = 512
H = 8
D = 64
KR = 64
SCALE = 0.125

P = 128
NT = S // P       # 16 tiles of 128 (queries and keys)
QC = 512          # query group width (psum bank)
NQG = S // QC     # 4 query groups
NC4 = DIM // P    # 4 contraction chunks

_CACHE = {}


# ----------------------------------------------------------------------
# Host-side planning: permutation + block structure from routes alone.

def _block_cost(pos, routes):
    qt = pos[: routes.shape[0]] // P
    rp = pos[routes] // P
    return len(np.unique(qt[:, None] * NT + rp))


def _seriate(routes):
    """Find perm (rho-position -> original index) making the route matrix
    banded. Barycenter sweeps; keeps the best block count seen."""
    routes = np.asarray(routes, np.int64)
    n, k = routes.shape
    qidx = np.repeat(np.arange(n), k)
    kidx = routes.ravel()
    best_perm = np.arange(n)
    best_cost = _block_cost(np.arange(n), routes)
    for variant in ("q", "qk"):
        pos = np.arange(n, dtype=np.float64)
        stale = 0
        for _ in range(48):
            bq = pos[routes].mean(1)
            if variant == "qk":
                sums = np.zeros(n)
                cnts = np.zeros(n)
                np.add.at(sums, kidx, pos[qidx])
                np.add.at(cnts, kidx, 1)
                bk = np.where(cnts > 0, sums / np.maximum(cnts, 1.0), pos)
                b = 0.5 * (bq + bk)
            else:
                b = bq
            order = np.argsort(b, kind="stable")
            npos = np.empty(n)
            npos[order] = np.arange(n)
            pos = npos
            c = _block_cost(pos.astype(np.int64), routes)
            if c < best_cost:
                best_cost, best_perm, stale = c, order.copy(), 0
            else:
                stale += 1
                if stale >= 6:
                    break
    return best_perm, best_cost


def plan_from_routes(routes):
    """-> (perm, plan, nblocks); plan[qt] = tuple of key tiles needed."""
    routes = np.asarray(routes, np.int64)
    perm, _ = _seriate(routes)
    pos = np.empty(S, np.int64)
    pos[perm] = np.arange(S)
    plan = []
    for qt in range(NT):
        qs = perm[qt * P:(qt + 1) * P]
        kts = np.unique(pos[routes[qs]] // P)
        plan.append(tuple(int(x) for x in kts))
    nblocks = sum(len(x) for x in plan)
    return perm, tuple(plan), nblocks


def _chunk4(seq):
    return [seq[i:i + 4] for i in range(0, len(seq), 4)]


# ----------------------------------------------------------------------
# Device program.

def build_nc(plan, nblocks):
    nc = bacc.Bacc(
        "TRN2",
        target_bir_lowering=False,
        debug=False,
        num_devices=8,
    )

    xt_d = nc.dram_tensor("xt", [P, NC4 * S], BF16, kind="ExternalInput").ap()
    wq_d = nc.dram_tensor("wq", [P, NC4 * P], BF16, kind="ExternalInput").ap()
    wk_d = nc.dram_tensor("wk", [P, NC4 * P], BF16, kind="ExternalInput").ap()
    wv_d = nc.dram_tensor("wv", [P, NC4 * P], BF16, kind="ExternalInput").ap()
    bq_d = nc.dram_tensor("bq", [P, 1], F32, kind="ExternalInput").ap()
    bk_d = nc.dram_tensor("bk", [P, 1], F32, kind="ExternalInput").ap()
    bv_d = nc.dram_tensor("bv", [P, 1], F32, kind="ExternalInput").ap()
    ct_d = nc.dram_tensor("ct", [P, nblocks * P], BF16, kind="ExternalInput").ap()
    wo_d = nc.dram_tensor("wo", [P, DIM], BF16, kind="ExternalInput").ap()
    out_d = nc.dram_tensor("out", [S, DIM], FP16, kind="ExternalOutput").ap()

    # enum offset of first block of each query tile
    boff = np.cumsum([0] + [len(x) for x in plan])

    with tile.TileContext(nc) as tc:
        with tc.tile_pool(name="persist", bufs=1) as pp:
            # count-mask blocks, packed per (qt, kt) enumeration
            ct_big = pp.tile([P, nblocks * P], BF16, tag="ctb", name="ct_big")
            nc.sync.dma_start(out=ct_big[:], in_=ct_d[:, :])

            xt_big = pp.tile([P, NC4 * S], BF16, tag="xtb", name="xt_big")
            for qc in range(NQG):
                nc.sync.dma_start(
                    out=xt_big[:, qc * 2048:(qc + 1) * 2048],
                    in_=xt_d[:, qc * 2048:(qc + 1) * 2048],
                )

            w_sb = {}
            for name, wd in (("q", wq_d), ("k", wk_d), ("v", wv_d)):
                wt = pp.tile([P, NC4 * P], BF16, tag=f"w{name}b", name=f"w{name}_big")
                nc.sync.dma_start(out=wt[:], in_=wd[:, :])
                for c in range(NC4):
                    w_sb[(name, c)] = wt[:, c * P:(c + 1) * P]
            b_sb = {}
            for name, bd in (("q", bq_d), ("k", bk_d), ("v", bv_d)):
                t = pp.tile([P, 1], F32, tag=f"b{name}", name=f"b{name}_sb")
                nc.sync.dma_start(out=t[:], in_=bd[:, :])
                b_sb[name] = t
            wo_sb = pp.tile([P, DIM], BF16, tag="wo")
            nc.sync.dma_start(out=wo_sb[:], in_=wo_d[:, :])

            sel_sb = {}
            for h in range(2):
                t = pp.tile([P, P], BF16, tag=f"sel{h}", name=f"sel{h}")
                nc.vector.memset(t[:], 0.0)
                nc.vector.memset(t[0:1, h * D:(h + 1) * D], 1.0)
                sel_sb[h] = t

            # q^T/k^T per-head, rows 64-127 zero-padded
            qkvt = {}
            for name in ("q", "k"):
                for h in range(2):
                    t = pp.tile([P, S], BF16, tag=f"{name}t{h}", name=f"{name}t{h}")
                    nc.vector.memset(t[D:P, :], 0.0)
                    qkvt[(name, h)] = t
            # V tiles in [key, d] layout with a ones column at col 64
            v_sb = {}
            for h in range(2):
                for kt in range(NT):
                    t = pp.tile([P, D + 1], BF16, tag=f"v{h}_{kt}", name=f"v{h}_{kt}")
                    nc.vector.memset(t[:, D:D + 1], 1.0)
                    v_sb[(h, kt)] = t

            ot_sb = pp.tile([P, S], F32, tag="ot")
            den_sb = {}
            for h in range(2):
                den_sb[h] = pp.tile([P, S], BF16, tag=f"den{h}", name=f"den{h}")
                nc.vector.memset(den_sb[h][:], 0.0)
            on_sb = pp.tile([P, S], BF16, tag="on")

            # Phase 1: per query group, K^T/Q^T (+bias) and V tiles.
            with tc.tile_pool(name="psum_pre", bufs=2, space="PSUM") as pre, \
                 tc.tile_pool(name="psum_v", bufs=2, space="PSUM") as vpl:
                for qc in range(NQG):
                    xq = xt_big[:, qc * 2048:(qc + 1) * 2048]
                    for name in ("k", "q"):
                        ps = pre.tile([P, QC], F32, tag="qkps", name="qkps")
                        for c in range(NC4):
                            nc.tensor.matmul(
                                ps[:],
                                lhsT=w_sb[(name, c)],
                                rhs=xq[:, c * QC:(c + 1) * QC],
                                start=(c == 0),
                                stop=(c == NC4 - 1),
                            )
                        for h in range(2):
                            hd = h * D
                            nc.vector.tensor_tensor(
                                out=qkvt[(name, h)][0:D, qc * QC:(qc + 1) * QC],
                                in0=ps[hd:hd + D, :],
                                in1=b_sb[name][hd:hd + D, :].to_broadcast([D, QC]),
                                op=mybir.AluOpType.add,
                            )
                    for kk in range(4):
                        kt = qc * 4 + kk
                        vp = vpl.tile([P, P], F32, tag="vps", name="vps")
                        for c in range(NC4):
                            nc.tensor.matmul(
                                vp[:],
                                lhsT=xq[:, c * QC + kk * P: c * QC + (kk + 1) * P],
                                rhs=w_sb[("v", c)],
                                start=(c == 0),
                                stop=(c == NC4 - 1),
                            )
                        nc.vector.tensor_copy(out=v_sb[(0, kt)][:, 0:D], in_=vp[:, 0:D])
                        nc.vector.tensor_copy(out=v_sb[(1, kt)][:, 0:D], in_=vp[:, D:P])

            # Phase 2 + 3: banded masked attention, then normalize+project,
            # pipelined per 512-query group.
            with tc.tile_pool(name="psum_s", bufs=2, space="PSUM") as spool, \
                 tc.tile_pool(name="psum_o", bufs=2, space="PSUM") as opool, \
                 tc.tile_pool(name="psum_r2", bufs=1, space="PSUM") as rpool, \
                 tc.tile_pool(name="psum_pr", bufs=2, space="PSUM") as prpool, \
                 tc.tile_pool(name="pwork", bufs=3) as pw, \
                 tc.tile_pool(name="fwork", bufs=2) as fw, \
                 tc.tile_pool(name="obuf", bufs=3) as ob:
                for qtg in range(NQG):
                    for h in range(2):
                        ops = opool.tile([P, QC], F32, tag="o", name="o_ps")
                        for qq in range(4):
                            qt = qtg * 4 + qq
                            kts = plan[qt]
                            groups = _chunk4(kts)
                            gi0 = 0
                            for g, grp in enumerate(groups):
                                w = len(grp) * P
                                sps = spool.tile([P, QC], F32, tag="s", name="s_ps")
                                for j, kt in enumerate(grp):
                                    nc.tensor.matmul(
                                        sps[:, j * P:(j + 1) * P],
                                        lhsT=qkvt[("k", h)][:, kt * P:(kt + 1) * P],
                                        rhs=qkvt[("q", h)][:, qt * P:(qt + 1) * P],
                                        start=True,
                                        stop=True,
                                    )
                                pb = pw.tile([P, QC], BF16, tag="p", name="p_sb")
                                nc.scalar.activation(
                                    pb[:, 0:w], sps[:, 0:w],
                                    mybir.ActivationFunctionType.Exp,
                                )
                                pm = pw.tile([P, QC], BF16, tag="pm", name="pm_sb")
                                co = (boff[qt] + gi0) * P
                                nc.vector.tensor_tensor(
                                    out=pm[:, 0:w],
                                    in0=pb[:, 0:w],
                                    in1=ct_big[:, co:co + w],
                                    op=mybir.AluOpType.mult,
                                )
                                for j, kt in enumerate(grp):
                                    nc.tensor.matmul(
                                        ops[0:D + 1, qq * P:(qq + 1) * P],
                                        lhsT=v_sb[(h, kt)][:],
                                        rhs=pm[:, j * P:(j + 1) * P],
                                        start=(g == 0 and j == 0),
                                        stop=(g == len(groups) - 1
                                              and j == len(grp) - 1),
                                    )
                                gi0 += len(grp)
                        hd = h * D
                        nc.vector.tensor_copy(
                            out=ot_sb[hd:hd + D, qtg * QC:(qtg + 1) * QC],
                            in_=ops[0:D, :],
                        )
                        nc.vector.tensor_copy(
                            out=den_sb[h][0:1, qtg * QC:(qtg + 1) * QC],
                            in_=ops[D:D + 1, :],
                        )
                    # Phase 3 for this query group.
                    qs = slice(qtg * QC, (qtg + 1) * QC)
                    r2 = rpool.tile([P, QC], F32, tag="r2", name="r2_ps")
                    for h in range(2):
                        nc.tensor.matmul(
                            r2[:],
                            lhsT=sel_sb[h][:],
                            rhs=den_sb[h][:, qs],
                            start=(h == 0),
                            stop=(h == 1),
                        )
                    rr = fw.tile([P, QC], F32, tag="rr", name="rr_sb")
                    nc.vector.reciprocal_approx_fast(out=rr[:], in_=r2[:])
                    tmp = fw.tile([P, QC], F32, tag="tmp", name="tmp_sb")
                    nc.vector.tensor_tensor(
                        out=tmp[:], in0=ot_sb[:, qs], in1=rr[:],
                        op=mybir.AluOpType.mult,
                    )
                    nc.vector.tensor_tensor(
                        out=on_sb[:, qs], in0=tmp[:],
                        in1=b_sb["v"][:].to_broadcast([P, QC]),
                        op=mybir.AluOpType.add,
                    )
                    for qq in range(4):
                        qt = qtg * 4 + qq
                        pr = prpool.tile([P, DIM], F32, tag="pr", name="pr_ps")
                        nc.tensor.matmul(
                            pr[:],
                            lhsT=on_sb[:, qt * P:(qt + 1) * P],
                            rhs=wo_sb[:],
                            start=True,
                            stop=True,
                        )
                        o16 = ob.tile([P, DIM], FP16, tag="o16", name="o16_sb")
                        nc.vector.tensor_copy(out=o16[:], in_=pr[:])
                        nc.sync.dma_start(
                            out=out_d[qt * P:(qt + 1) * P, :], in_=o16[:]
                        )

    nc.compile()
    return nc


def prepare(routes):
    routes = np.asarray(routes)
    key = routes.tobytes()
    if _CACHE.get("key") == key:
        return _CACHE["nc"], _CACHE["perm"], _CACHE["plan"], _CACHE["nblocks"]
    perm, plan, nblocks = plan_from_routes(routes)
    nc = build_nc(plan, nblocks)
    _CACHE.update(key=key, nc=nc, perm=perm, plan=plan, nblocks=nblocks)
    return nc, perm, plan, nblocks


# ----------------------------------------------------------------------
# Host-side data marshalling.

def make_in_maps(x, routes, w_qkv, b_qkv, w_out, perm, plan, nblocks):
    x = np.asarray(x, np.float32)
    routes = np.asarray(routes)
    w_qkv = np.asarray(w_qkv, np.float32)
    b_qkv = np.asarray(b_qkv, np.float32)
    w_out = np.asarray(w_out, np.float32)

    # count matrix in permuted space, packed per-block [keys, queries]
    C = np.zeros((S, S), np.float32)
    np.add.at(C, (np.arange(S)[:, None], routes), 1.0)
    Cp = C[np.ix_(perm, perm)]          # [q-pos, k-pos]
    blocks = []
    for qt in range(NT):
        for kt in plan[qt]:
            blocks.append(np.ascontiguousarray(
                Cp[qt * P:(qt + 1) * P, kt * P:(kt + 1) * P].T))
    ctp = np.concatenate(blocks, axis=1).astype(NPBF16)
    assert ctp.shape == (P, nblocks * P)

    def pack(a):
        # [n*128, X] -> [128, n*X]
        n = a.shape[0] // P
        return np.ascontiguousarray(
            a.reshape(n, P, a.shape[1]).transpose(1, 0, 2).reshape(P, -1))

    # x^T permuted, query-group-major: [128, (qc, c) blocks of 512]
    xts = []
    for b in range(B):
        xpT = np.ascontiguousarray(x[b][perm].T)        # [512, 2048]
        cols = []
        for qc in range(NQG):
            for c in range(NC4):
                cols.append(xpT[c * P:(c + 1) * P, qc * QC:(qc + 1) * QC])
        xts.append(np.concatenate(cols, axis=1).astype(NPBF16))

    in_maps = []
    for core in range(8):
        b = core // 4
        hp = core % 4
        col = hp * P
        wq = pack(w_qkv[:, col:col + P] * SCALE).astype(NPBF16)
        wk = pack(w_qkv[:, DIM + col:DIM + col + P]).astype(NPBF16)
        wv = pack(w_qkv[:, 2 * DIM + col:2 * DIM + col + P]).astype(NPBF16)
        bq = (b_qkv[col:col + P] * SCALE).astype(np.float32).reshape(P, 1)
        bk = b_qkv[DIM + col:DIM + col + P].astype(np.float32).reshape(P, 1)
        bv = b_qkv[2 * DIM + col:2 * DIM + col + P].astype(np.float32).reshape(P, 1)
        wo = np.ascontiguousarray(w_out[col:col + P, :]).astype(NPBF16)
        in_maps.append(dict(
            xt=xts[b], wq=wq, wk=wk, wv=wv, bq=bq, bk=bk, bv=bv,
            ct=ctp, wo=wo,
        ))
    return in_maps


def run(inputs, trace=False, trace_cores=None):
    nc, perm, plan, nblocks = prepare(inputs["routes"])
    in_maps = make_in_maps(
        inputs["x"], inputs["routes"], inputs["w_qkv"], inputs["b_qkv"],
        inputs["w_out"], perm, plan, nblocks,
    )
    res = run_bass_kernel_spmd(
        nc, in_maps, list(range(8)), trace=trace, trace_cores=trace_cores,
    )
    b_out = np.asarray(inputs["b_out"], np.float32)
    final = np.zeros((B, S, DIM), np.float32)
    for core in range(8):
        final[core // 4][perm] += np.asarray(
            res.results[core]["out"], np.float32)
    final += b_out[None, None, :]
    return final, res


def kernel(**inputs):
    final, _ = run(inputs, trace=False)
    return final


# revision 4
# speedup vs baseline: 1.9262x; 1.2007x over previous
"""CantorAttention Trainium2 kernel — banded block-sparse edition.

Problem (hardcoded): B=2, S=2048, DIM=512, H=8 heads, D=64, K=64 routes.
  qkv = x @ w_qkv + b_qkv ; per-head sparse attention over routes[q, :] ;
  out = attn_out @ w_out + b_out.

Strategy (8 cores): shard batch x head-pairs. Core i handles batch i//4 and
heads (2*(i%4), 2*(i%4)+1). Routes are shared across batch/heads.

The sparse attention is made BANDED by a host-computed permutation rho of
the sequence (barycenter seriation of the bipartite route graph — for the
Cantor routes this recovers coordinate order and every query's 64 routes
land in a <=128-wide window of permuted key space). Queries and keys are
both processed in rho order; only the nonzero 128x128 (key-tile x
query-tile) blocks of the routed score matrix are computed:

  P[k, q]  = C~[k, q] * exp(scale * (k_vec . q_vec))   (count mask, 0 off-route)
  AV_h     = [V_h | 1]^T @ P      -> rows 0..63 out, row 64 = denominator
  out_h    = AV_h / denom + bv_h  ;  partial = concat_h(out_h)^T @ w_out
Host gathers: final[b][perm] += partial_core ; final += b_out.

Exact softmax over the 64 routed scores for ANY routes input (block list
is derived from the actual routes; a bad permutation only costs speed).
All matmuls bf16 with fp32 PSUM accumulation; exp on ScalarE; mask-multiply
on VectorE; no on-chip transposes anywhere (V is produced directly in
[key, d] layout by using x^T blocks as the stationary operand).
"""

import numpy as np
import ml_dtypes

import concourse.bass as bass
import concourse.bacc as bacc
import concourse.mybir as mybir
import concourse.tile as tile
from concourse.bass_utils import run_bass_kernel_spmd

BF16 = mybir.dt.bfloat16
F32 = mybir.dt.float32
FP16 = mybir.dt.float16
NPBF16 = ml_dtypes.bfloat16

B = 2
S = 2048
DIM = 512
H = 8
D = 64
KR = 64
SCALE = 0.125

P = 128
NT = S // P       # 16 tiles of 128 (queries and keys)
QC = 512          # query group width (psum bank)
NQG = S // QC     # 4 query groups
NC4 = DIM // P    # 4 contraction chunks

_CACHE = {}


# ----------------------------------------------------------------------
# Host-side planning: permutation + block structure from routes alone.

def _block_cost(pos, routes):
    qt = pos[: routes.shape[0]] // P
    rp = pos[routes] // P
    return len(np.unique(qt[:, None] * NT + rp))


def _seriate(routes):
    """Find perm (rho-position -> original index) making the route matrix
    banded. Barycenter sweeps; keeps the best block count seen."""
    routes = np.asarray(routes, np.int64)
    n, k = routes.shape
    qidx = np.repeat(np.arange(n), k)
    kidx = routes.ravel()
    best_perm = np.arange(n)
    best_cost = _block_cost(np.arange(n), routes)
    for variant in ("q", "qk"):
        pos = np.arange(n, dtype=np.float64)
        stale = 0
        for _ in range(48):
            bq = pos[routes].mean(1)
            if variant == "qk":
                sums = np.zeros(n)
                cnts = np.zeros(n)
                np.add.at(sums, kidx, pos[qidx])
                np.add.at(cnts, kidx, 1)
                bk = np.where(cnts > 0, sums / np.maximum(cnts, 1.0), pos)
                b = 0.5 * (bq + bk)
            else:
                b = bq
            order = np.argsort(b, kind="stable")
            npos = np.empty(n)
            npos[order] = np.arange(n)
            pos = npos
            c = _block_cost(pos.astype(np.int64), routes)
            if c < best_cost:
                best_cost, best_perm, stale = c, order.copy(), 0
            else:
                stale += 1
                if stale >= 6:
                    break
    return best_perm, best_cost


def plan_from_routes(routes):
    """-> (perm, plan, nblocks); plan[qt] = tuple of key tiles needed."""
    routes = np.asarray(routes, np.int64)
    perm, _ = _seriate(routes)
    pos = np.empty(S, np.int64)
    pos[perm] = np.arange(S)
    plan = []
    for qt in range(NT):
        qs = perm[qt * P:(qt + 1) * P]
        kts = np.unique(pos[routes[qs]] // P)
        plan.append(tuple(int(x) for x in kts))
    nblocks = sum(len(x) for x in plan)
    return perm, tuple(plan), nblocks


def _chunk4(seq):
    return [seq[i:i + 4] for i in range(0, len(seq), 4)]


# ----------------------------------------------------------------------
# Device program.

def build_nc(plan, nblocks):
    nc = bacc.Bacc(
        "TRN2",
        target_bir_lowering=False,
        debug=False,
        num_devices=8,
    )

    xt_d = nc.dram_tensor("xt", [P, NC4 * S], BF16, kind="ExternalInput").ap()
    wq_d = nc.dram_tensor("wq", [P, NC4 * P], BF16, kind="ExternalInput").ap()
    wk_d = nc.dram_tensor("wk", [P, NC4 * P], BF16, kind="ExternalInput").ap()
    wv_d = nc.dram_tensor("wv", [P, NC4 * P], BF16, kind="ExternalInput").ap()
    bq_d = nc.dram_tensor("bq", [P, 1], F32, kind="ExternalInput").ap()
    bk_d = nc.dram_tensor("bk", [P, 1], F32, kind="ExternalInput").ap()
    bv_d = nc.dram_tensor("bv", [P, 1], F32, kind="ExternalInput").ap()
    ct_d = nc.dram_tensor("ct", [P, nblocks * P], BF16, kind="ExternalInput").ap()
    wo_d = nc.dram_tensor("wo", [P, DIM], BF16, kind="ExternalInput").ap()
    out_d = nc.dram_tensor("out", [S, DIM], FP16, kind="ExternalOutput").ap()

    # enum offset of first block of each query tile
    boff = np.cumsum([0] + [len(x) for x in plan])

    with tile.TileContext(nc) as tc:
        with tc.tile_pool(name="persist", bufs=1) as pp:
            # DMAs in critical-path order: weights/biases first (first
            # matmuls need them), then x chunks, then the mask blocks.
            w_sb = {}
            wts = {}
            for name, wd in (("q", wq_d), ("k", wk_d), ("v", wv_d)):
                wt = pp.tile([P, NC4 * P], BF16, tag=f"w{name}b", name=f"w{name}_big")
                nc.sync.dma_start(out=wt[:], in_=wd[:, :])
                wts[name] = wt
                for c in range(NC4):
                    w_sb[(name, c)] = wt[:, c * P:(c + 1) * P]
            b_sb = {}
            for name, bd in (("q", bq_d), ("k", bk_d), ("v", bv_d)):
                t = pp.tile([P, 1], F32, tag=f"b{name}", name=f"b{name}_sb")
                nc.sync.dma_start(out=t[:], in_=bd[:, :])
                b_sb[name] = t

            xt_big = pp.tile([P, NC4 * S], BF16, tag="xtb", name="xt_big")
            for qc in range(NQG):
                nc.sync.dma_start(
                    out=xt_big[:, qc * 2048:(qc + 1) * 2048],
                    in_=xt_d[:, qc * 2048:(qc + 1) * 2048],
                )

            # count-mask blocks, packed per (qt, kt); split per query group
            ct_big = pp.tile([P, nblocks * P], BF16, tag="ctb", name="ct_big")
            for g in range(NQG):
                lo, hi = boff[4 * g] * P, boff[4 * (g + 1)] * P
                nc.sync.dma_start(out=ct_big[:, lo:hi], in_=ct_d[:, lo:hi])

            wo_sb = pp.tile([P, DIM], BF16, tag="wo")
            nc.sync.dma_start(out=wo_sb[:], in_=wo_d[:, :])

            sel_sb = {}
            for h in range(2):
                t = pp.tile([P, P], BF16, tag=f"sel{h}", name=f"sel{h}")
                nc.gpsimd.memset(t[:], 0.0)
                nc.gpsimd.memset(t[0:1, h * D:(h + 1) * D], 1.0)
                sel_sb[h] = t

            # q^T/k^T per-head, rows 64-127 zero-padded
            qkvt = {}
            for name in ("q", "k"):
                for h in range(2):
                    t = pp.tile([P, S], BF16, tag=f"{name}t{h}", name=f"{name}t{h}")
                    nc.gpsimd.memset(t[D:P, :], 0.0)
                    qkvt[(name, h)] = t
            # V tiles in [key, d] layout with a ones column at col 64
            v_sb = {}
            for h in range(2):
                for kt in range(NT):
                    t = pp.tile([P, D + 1], BF16, tag=f"v{h}_{kt}", name=f"v{h}_{kt}")
                    nc.gpsimd.memset(t[:, D:D + 1], 1.0)
                    v_sb[(h, kt)] = t

            den_sb = {}
            for h in range(2):
                den_sb[h] = pp.tile([P, S], BF16, tag=f"den{h}", name=f"den{h}")
                nc.gpsimd.memset(den_sb[h][:], 0.0)
            on_sb = pp.tile([P, S], BF16, tag="on")

            # Phase 1: per query group, K^T/Q^T (+bias) and V tiles.
            with tc.tile_pool(name="psum_pre", bufs=2, space="PSUM") as pre, \
                 tc.tile_pool(name="psum_v", bufs=2, space="PSUM") as vpl:
                for qc in range(NQG):
                    xq = xt_big[:, qc * 2048:(qc + 1) * 2048]
                    for name in ("k", "q"):
                        ps = pre.tile([P, QC], F32, tag="qkps", name="qkps")
                        for c in range(NC4):
                            nc.tensor.matmul(
                                ps[:],
                                lhsT=w_sb[(name, c)],
                                rhs=xq[:, c * QC:(c + 1) * QC],
                                start=(c == 0),
                                stop=(c == NC4 - 1),
                            )
                        for h in range(2):
                            hd = h * D
                            nc.vector.tensor_tensor(
                                out=qkvt[(name, h)][0:D, qc * QC:(qc + 1) * QC],
                                in0=ps[hd:hd + D, :],
                                in1=b_sb[name][hd:hd + D, :].to_broadcast([D, QC]),
                                op=mybir.AluOpType.add,
                            )
                    for kk in range(4):
                        kt = qc * 4 + kk
                        vp = vpl.tile([P, P], F32, tag="vps", name="vps")
                        for c in range(NC4):
                            nc.tensor.matmul(
                                vp[:],
                                lhsT=xq[:, c * QC + kk * P: c * QC + (kk + 1) * P],
                                rhs=w_sb[("v", c)],
                                start=(c == 0),
                                stop=(c == NC4 - 1),
                            )
                        nc.any.tensor_copy(out=v_sb[(0, kt)][:, 0:D], in_=vp[:, 0:D])
                        nc.any.tensor_copy(out=v_sb[(1, kt)][:, 0:D], in_=vp[:, D:P])

            # Phase 2 + 3: banded masked attention, then normalize+project,
            # pipelined per 512-query group.
            with tc.tile_pool(name="psum_s", bufs=2, space="PSUM") as spool, \
                 tc.tile_pool(name="psum_o", bufs=3, space="PSUM") as opool, \
                 tc.tile_pool(name="psum_r2", bufs=1, space="PSUM") as rpool, \
                 tc.tile_pool(name="psum_pr", bufs=2, space="PSUM") as prpool, \
                 tc.tile_pool(name="pwork", bufs=3) as pw, \
                 tc.tile_pool(name="fwork", bufs=2) as fw, \
                 tc.tile_pool(name="obuf", bufs=3) as ob:
                for qtg in range(NQG):
                    o_tiles = {}
                    for h in range(2):
                        ops = opool.tile([P, QC], F32, tag="o", name="o_ps")
                        o_tiles[h] = ops
                        for qq in range(4):
                            qt = qtg * 4 + qq
                            kts = plan[qt]
                            groups = _chunk4(kts)
                            gi0 = 0
                            for g, grp in enumerate(groups):
                                w = len(grp) * P
                                sps = spool.tile([P, QC], F32, tag="s", name="s_ps")
                                for j, kt in enumerate(grp):
                                    nc.tensor.matmul(
                                        sps[:, j * P:(j + 1) * P],
                                        lhsT=qkvt[("k", h)][:, kt * P:(kt + 1) * P],
                                        rhs=qkvt[("q", h)][:, qt * P:(qt + 1) * P],
                                        start=True,
                                        stop=True,
                                    )
                                pb = pw.tile([P, QC], BF16, tag="p", name="p_sb")
                                nc.scalar.activation(
                                    pb[:, 0:w], sps[:, 0:w],
                                    mybir.ActivationFunctionType.Exp,
                                )
                                pm = pw.tile([P, QC], BF16, tag="pm", name="pm_sb")
                                co = (boff[qt] + gi0) * P
                                nc.vector.tensor_tensor(
                                    out=pm[:, 0:w],
                                    in0=pb[:, 0:w],
                                    in1=ct_big[:, co:co + w],
                                    op=mybir.AluOpType.mult,
                                )
                                for j, kt in enumerate(grp):
                                    nc.tensor.matmul(
                                        ops[0:D + 1, qq * P:(qq + 1) * P],
                                        lhsT=v_sb[(h, kt)][:],
                                        rhs=pm[:, j * P:(j + 1) * P],
                                        start=(g == 0 and j == 0),
                                        stop=(g == len(groups) - 1
                                              and j == len(grp) - 1),
                                    )
                                gi0 += len(grp)
                        nc.vector.tensor_copy(
                            out=den_sb[h][0:1, qtg * QC:(qtg + 1) * QC],
                            in_=ops[D:D + 1, :],
                        )
                    # Phase 3 for this query group.
                    qs = slice(qtg * QC, (qtg + 1) * QC)
                    r2 = rpool.tile([P, QC], F32, tag="r2", name="r2_ps")
                    for h in range(2):
                        nc.tensor.matmul(
                            r2[:],
                            lhsT=sel_sb[h][:],
                            rhs=den_sb[h][:, qs],
                            start=(h == 0),
                            stop=(h == 1),
                        )
                    rr = fw.tile([P, QC], F32, tag="rr", name="rr_sb")
                    nc.vector.reciprocal_approx_fast(out=rr[:], in_=r2[:])
                    tmp = fw.tile([P, QC], F32, tag="tmp", name="tmp_sb")
                    for h in range(2):
                        hd = h * D
                        nc.vector.tensor_tensor(
                            out=tmp[hd:hd + D, :],
                            in0=o_tiles[h][0:D, :],
                            in1=rr[hd:hd + D, :],
                            op=mybir.AluOpType.mult,
                        )
                    nc.vector.tensor_tensor(
                        out=on_sb[:, qs], in0=tmp[:],
                        in1=b_sb["v"][:].to_broadcast([P, QC]),
                        op=mybir.AluOpType.add,
                    )
                    for qq in range(4):
                        qt = qtg * 4 + qq
                        pr = prpool.tile([P, DIM], F32, tag="pr", name="pr_ps")
                        nc.tensor.matmul(
                            pr[:],
                            lhsT=on_sb[:, qt * P:(qt + 1) * P],
                            rhs=wo_sb[:],
                            start=True,
                            stop=True,
                        )
                        o16 = ob.tile([P, DIM], FP16, tag="o16", name="o16_sb")
                        nc.any.tensor_copy(out=o16[:], in_=pr[:])
                        nc.sync.dma_start(
                            out=out_d[qt * P:(qt + 1) * P, :], in_=o16[:]
                        )

    nc.compile()
    return nc


def prepare(routes):
    routes = np.asarray(routes)
    key = routes.tobytes()
    if _CACHE.get("key") == key:
        return _CACHE["nc"], _CACHE["perm"], _CACHE["plan"], _CACHE["nblocks"]
    perm, plan, nblocks = plan_from_routes(routes)
    nc = build_nc(plan, nblocks)
    _CACHE.update(key=key, nc=nc, perm=perm, plan=plan, nblocks=nblocks)
    return nc, perm, plan, nblocks


# ----------------------------------------------------------------------
# Host-side data marshalling.

def make_in_maps(x, routes, w_qkv, b_qkv, w_out, perm, plan, nblocks):
    x = np.asarray(x, np.float32)
    routes = np.asarray(routes)
    w_qkv = np.asarray(w_qkv, np.float32)
    b_qkv = np.asarray(b_qkv, np.float32)
    w_out = np.asarray(w_out, np.float32)

    # count matrix in permuted space, packed per-block [keys, queries]
    C = np.zeros((S, S), np.float32)
    np.add.at(C, (np.arange(S)[:, None], routes), 1.0)
    Cp = C[np.ix_(perm, perm)]          # [q-pos, k-pos]
    blocks = []
    for qt in range(NT):
        for kt in plan[qt]:
            blocks.append(np.ascontiguousarray(
                Cp[qt * P:(qt + 1) * P, kt * P:(kt + 1) * P].T))
    ctp = np.concatenate(blocks, axis=1).astype(NPBF16)
    assert ctp.shape == (P, nblocks * P)

    def pack(a):
        # [n*128, X] -> [128, n*X]
        n = a.shape[0] // P
        return np.ascontiguousarray(
            a.reshape(n, P, a.shape[1]).transpose(1, 0, 2).reshape(P, -1))

    # x^T permuted, query-group-major: [128, (qc, c) blocks of 512]
    xts = []
    for b in range(B):
        xpT = np.ascontiguousarray(x[b][perm].T)        # [512, 2048]
        cols = []
        for qc in range(NQG):
            for c in range(NC4):
                cols.append(xpT[c * P:(c + 1) * P, qc * QC:(qc + 1) * QC])
        xts.append(np.concatenate(cols, axis=1).astype(NPBF16))

    in_maps = []
    for core in range(8):
        b = core // 4
        hp = core % 4
        col = hp * P
        wq = pack(w_qkv[:, col:col + P] * SCALE).astype(NPBF16)
        wk = pack(w_qkv[:, DIM + col:DIM + col + P]).astype(NPBF16)
        wv = pack(w_qkv[:, 2 * DIM + col:2 * DIM + col + P]).astype(NPBF16)
        bq = (b_qkv[col:col + P] * SCALE).astype(np.float32).reshape(P, 1)
        bk = b_qkv[DIM + col:DIM + col + P].astype(np.float32).reshape(P, 1)
        bv = b_qkv[2 * DIM + col:2 * DIM + col + P].astype(np.float32).reshape(P, 1)
        wo = np.ascontiguousarray(w_out[col:col + P, :]).astype(NPBF16)
        in_maps.append(dict(
            xt=xts[b], wq=wq, wk=wk, wv=wv, bq=bq, bk=bk, bv=bv,
            ct=ctp, wo=wo,
        ))
    return in_maps


def run(inputs, trace=False, trace_cores=None):
    nc, perm, plan, nblocks = prepare(inputs["routes"])
    in_maps = make_in_maps(
        inputs["x"], inputs["routes"], inputs["w_qkv"], inputs["b_qkv"],
        inputs["w_out"], perm, plan, nblocks,
    )
    res = run_bass_kernel_spmd(
        nc, in_maps, list(range(8)), trace=trace, trace_cores=trace_cores,
    )
    b_out = np.asarray(inputs["b_out"], np.float32)
    final = np.zeros((B, S, DIM), np.float32)
    for core in range(8):
        final[core // 4][perm] += np.asarray(
            res.results[core]["out"], np.float32)
    final += b_out[None, None, :]
    return final, res


def kernel(**inputs):
    final, _ = run(inputs, trace=False)
    return final
